# revision 16
# baseline (speedup 1.0000x reference)
"""Causal ConvTranspose1d (grouped, stride 8) Trainium2 Bass kernel.

Problem (hardcoded):
  x      [8, 512, 4096]  f32
  weight [512, 16, 1]    f32
  bias   [256]           f32
  out    [8, 256, 32768] f32   (= [B, Cout, T*stride])

Math (derived from the reference grouped dilated conv):
  with w2 = weight.reshape(512, 16), cpg = 2, stride = 8, K = 16:
  y[b, co, 8*t + r] = sum_{j in 0..1} ( w2[2co+j, r]   * x[b, 2co+j, t]
                                      + w2[2co+j, r+8] * x[b, 2co+j, t-1] )
                      + bias[co]          (x[., -1] == 0)

Sharding: data-parallel over batch; one batch element per NeuronCore (8 cores).

Variant "g2" (default):
  - x is loaded bf16 in natural 128-channel slices q = 0..3 ([128, 1+4096],
    leading zero column for the t-1 tap).
  - Pair-packed stationary matrices [128, 128] bf16 with 2 nonzeros per
    column: column 64*u + co_l accumulates both input channels (j = 0, 1)
    of output channel 64*q + co_l for phase r = m + 4*u.  One matmul per
    (m, tap) covers both j terms -> half the PE column streams of the
    diagonal formulation.
  - Matmuls write PSUM with free-dim stride 4 (offset m), so a PSUM bank
    [128 = (u, co_l), 512 = (t, m)] holds 128 t x 8 phases with phases
    m = 0..3 interleaved; the drain copies then write 16B-contiguous runs
    (4 f32) per output-time step instead of isolated 4B elements.
  - Drain copies (+ bias add) are split across Scalar, Vector and GpSimd.
  - [64, 4096] f32 staging tiles are DMAed to HBM contiguously.

Variant "h": diagonal stationaries (1 nonzero/col, 128-wide PSUM,
phase-major) as the previous baseline, but bf16 inputs and the drain
copies split across all three engines.
"""

import numpy as np

B, CIN, COUT, K, T = 8, 512, 256, 16, 4096
STRIDE = 8
SOUT = T * STRIDE  # 32768
NCORES = 8
VARIANT = "qp4p"

_CACHE = {}


def _build_nc(variant=None):
    import concourse.mybir as mybir
    from concourse import bacc
    from concourse.tile import TileContext

    f32 = mybir.dt.float32
    bf16 = mybir.dt.bfloat16
    variant = variant or VARIANT

    nc = bacc.Bacc(trn_type="TRN2", target_bir_lowering=False, debug=False)

    if variant in ("qp4", "qp4p"):
        # 64 pair stationaries [128, 64]: blk = (q*8 + r)*2 + tap
        WDCOLS = 64 * 64
    elif variant == "g2":
        # 32 pair stationaries: blk = (q*4 + m)*2 + tap, each [128, 128]
        WDCOLS = 32 * 128
    else:
        # 64 diag stationaries: blk = ((ct*2 + j)*16 + k), each [128, 128]
        WDCOLS = 64 * 128
    x = nc.dram_tensor("x", [CIN, 1 + T], bf16, kind="ExternalInput").ap()
    wd = nc.dram_tensor("wd", [128, WDCOLS], bf16, kind="ExternalInput").ap()
    bias = nc.dram_tensor("bias", [128, 4], f32, kind="ExternalInput").ap()
    y = nc.dram_tensor("y", [COUT, SOUT], f32, kind="ExternalOutput").ap()

    # weighted round-robin over the two PSUM-capable copy engines
    # (GpSimd/Pool cannot access PSUM).  Act @1.2GHz vs DVE @0.96GHz
    # -> 5:4 split balances their busy time.
    def engines(nc):
        pat = [nc.scalar, nc.vector] * 4 + [nc.scalar]

        def pick(i):
            return pat[i % len(pat)]

        return pick

    with TileContext(nc) as tc:
        ps_bufs = 1 if variant in ("qp4", "qp4p") else 2
        y_bufs = 2 if variant in ("qp4", "qp4p") else 3
        with (
            tc.tile_pool(name="const", bufs=1) as cpool,
            tc.tile_pool(name="xp", bufs=2) as xpool,
            tc.tile_pool(name="yp", bufs=y_bufs) as ypool,
            tc.tile_pool(name="ps", bufs=ps_bufs, space="PSUM") as pspool,
        ):
            wd_t = cpool.tile([128, WDCOLS], bf16)
            nc.sync.dma_start(out=wd_t, in_=wd)
            bias_t = cpool.tile([128, 4], f32)
            nc.sync.dma_start(out=bias_t, in_=bias)
            pick = engines(nc)

            if variant in ("qp4", "qp4p"):
                _emit_qp4(
                    nc, tc, xpool, ypool, pspool, x, y, wd_t, bias_t, pick,
                    preload=(variant == "qp4p"),
                )
            elif variant == "g2":
                _emit_g2(nc, tc, xpool, ypool, pspool, x, y, wd_t, bias_t, pick)
            else:
                _emit_h(nc, tc, xpool, ypool, pspool, x, y, wd_t, bias_t, pick)
    nc.compile()
    return nc


def _emit_qp4(nc, tc, xpool, ypool, pspool, x, y, wd_t, bias_t, pick, preload=False):
    """q-pair col-tiled variant: PSUM partitions = 128 contiguous co.

    Two M=64 matmuls per (r, tap) — one per q-slice of the co-pair — at
    tile_position (0, 0) and (0, 64) fill a [128, 512] PSUM bank that
    holds 128 t x 4 phases (free = 4*tl + (r - 4h)).  Copies are 128-wide
    with 16B-contiguous runs; y staging spans 128 partitions so the DMA
    reads at full rate.
    """
    import concourse.mybir as mybir

    f32 = mybir.dt.float32
    bf16 = mybir.dt.bfloat16
    TCH = 128            # t per PSUM bank
    NSG = 4              # staging groups per co-pair
    SGT = T // NSG       # 1024 t per staging group
    NB = SGT // TCH      # 8 banks per (staging group, r-half)
    ci = 0
    for g in range(2):
        xg = []
        for gh in range(2):
            q = 2 * g + gh
            x_t = xpool.tile([128, 1 + T], bf16, tag=f"x{gh}", name=f"x_t{gh}")
            nc.sync.dma_start(out=x_t, in_=x[128 * q : 128 * (q + 1), :])
            xg.append(x_t)
        for sg in range(NSG):
            y_t = ypool.tile([128, STRIDE * SGT], f32, tag="y", name="y_t")
            y_r = y_t.rearrange("p (t e) -> p t e", e=STRIDE)
            # Bank-outer order: each bank finishes its 16 matmuls
            # consecutively so its drain copy overlaps the next bank's
            # matmuls instead of all 8 banks completing at once.
            for b in range(NB):
                t0 = (sg * NB + b) * TCH
                for h in range(2):
                    p_t = pspool.tile(
                        [128, 4 * TCH], f32,
                        tag=f"ps{b % 4}_{h}", name=f"p_t{b % 4}_{h}",
                    )
                    for rr in range(4):
                        r = 4 * h + rr
                        for tap in range(2):
                            mms = []
                            for gh in range(2):
                                blk = ((2 * g + gh) * 8 + r) * 2 + tap
                                lhsT = wd_t[:, blk * 64 : (blk + 1) * 64]
                                if preload:
                                    nc.tensor.ldweights(
                                        lhsT, tile_position=(0, 64 * gh)
                                    )
                            for gh in range(2):
                                blk = ((2 * g + gh) * 8 + r) * 2 + tap
                                lhsT = wd_t[:, blk * 64 : (blk + 1) * 64]
                                rhs = xg[gh][
                                    :, (1 - tap) + t0 : (1 - tap) + t0 + TCH
                                ]
                                mm = nc.tensor.matmul(
                                    p_t[64 * gh : 64 * (gh + 1), rr : 4 * TCH : 4],
                                    lhsT,
                                    rhs,
                                    start=(tap == 0),
                                    stop=(tap == 1),
                                    tile_position=(0, 64 * gh),
                                )
                                if preload:
                                    mm.ins.ldweights = False
                    dst = y_r[:, TCH * b : TCH * (b + 1), 4 * h : 4 * h + 4]
                    b_ap = bias_t[:, g : g + 1]
                    eng = pick(ci)
                    ci += 1
                    if eng is nc.scalar:
                        eng.add(dst, p_t, b_ap)
                    else:
                        eng.tensor_scalar_add(dst, p_t, b_ap)
            nc.sync.dma_start(
                out=y[
                    128 * g : 128 * (g + 1),
                    STRIDE * SGT * sg : STRIDE * SGT * (sg + 1),
                ],
                in_=y_t,
            )


def _emit_g2(nc, tc, xpool, ypool, pspool, x, y, wd_t, bias_t, pick):
    import concourse.mybir as mybir

    f32 = mybir.dt.float32
    bf16 = mybir.dt.bfloat16
    TCH = 128            # t per PSUM bank
    NB = T // TCH        # 32 banks' worth of t-chunks per q
    BPS = 4              # banks per staging tile ([64, 4096] = 512 t)
    ci = 0
    for q in range(4):
        x_t = xpool.tile([128, 1 + T], bf16, tag="x", name="x_t")
        nc.sync.dma_start(out=x_t, in_=x[128 * q : 128 * (q + 1), :])
        for sg in range(NB // BPS):  # staging groups of 4 banks
            y_t = ypool.tile([64, STRIDE * TCH * BPS], f32, tag="y", name="y_t")
            p_ts = []
            for b in range(BPS):
                t0 = (sg * BPS + b) * TCH
                p_t = pspool.tile([128, 4 * TCH], f32, tag=f"ps{b}", name=f"p_t{b}")
                p_ts.append(p_t)
                for m in range(4):
                    blk = (q * 4 + m) * 2
                    for tap in range(2):
                        rhs = x_t[:, (1 - tap) + t0 : (1 - tap) + t0 + TCH]
                        nc.tensor.matmul(
                            p_t[:, m : 4 * TCH : 4],
                            wd_t[:, (blk + tap) * 128 : (blk + tap + 1) * 128],
                            rhs,
                            start=(tap == 0),
                            stop=(tap == 1),
                        )
            y_r = y_t.rearrange("p (t e) -> p t e", e=STRIDE)
            for b in range(BPS):
                for u in range(2):
                    # P[64u + co_l, 4*tl + m] -> y_t[co_l, 8*tl + 4u + m]
                    dst = y_r[:, TCH * b : TCH * (b + 1), 4 * u : 4 * u + 4]
                    src = p_ts[b][64 * u : 64 * (u + 1), :]
                    b_ap = bias_t[64 * u : 64 * (u + 1), q : q + 1]
                    eng = pick(ci)
                    ci += 1
                    if eng is nc.scalar:
                        eng.add(dst, src, b_ap)
                    else:
                        eng.tensor_scalar_add(dst, src, b_ap)
            nc.sync.dma_start(
                out=y[
                    64 * q : 64 * (q + 1),
                    STRIDE * TCH * BPS * sg : STRIDE * TCH * BPS * (sg + 1),
                ],
                in_=y_t,
            )


def _emit_h(nc, tc, xpool, ypool, pspool, x, y, wd_t, bias_t, pick):
    import concourse.mybir as mybir

    f32 = mybir.dt.float32
    bf16 = mybir.dt.bfloat16
    TWIN = 512
    NTWIN = T // TWIN  # 8
    ci = 0
    for ct in range(2):
        xj = []
        for j in range(2):
            x_t = xpool.tile([128, 1 + T], bf16, tag=f"x{j}", name=f"x_t{j}")
            src = x[256 * ct + j : 256 * ct + 256 : 2, :]
            nc.sync.dma_start(out=x_t, in_=src)
            xj.append(x_t)
        for twin in range(NTWIN):
            t0 = twin * TWIN
            y_t = ypool.tile([128, STRIDE * TWIN], f32, tag="y", name="y_t")
            for r in range(8):
                p_t = pspool.tile(
                    [128, TWIN], f32, tag=f"ps{r % 4}", name=f"p_t{r % 4}"
                )
                for j in range(2):
                    for tap in range(2):
                        k = r + 8 * tap
                        rhs = xj[j][:, (1 - tap) + t0 : (1 - tap) + t0 + TWIN]
                        col = ((ct * 2 + j) * 16 + k) * 128
                        nc.tensor.matmul(
                            p_t,
                            wd_t[:, col : col + 128],
                            rhs,
                            start=(tap == 0 and j == 0),
                            stop=(tap == 1 and j == 1),
                        )
                out_ap = y_t[:, r : STRIDE * TWIN : STRIDE]
                b_ap = bias_t[:, 2 * ct : 2 * ct + 1]
                eng = pick(ci)
                ci += 1
                if eng is nc.scalar:
                    eng.add(out_ap, p_t, b_ap)
                else:
                    eng.tensor_scalar_add(out_ap, p_t, b_ap)
            nc.sync.dma_start(
                out=y[
                    128 * ct : 128 * (ct + 1),
                    STRIDE * t0 : STRIDE * t0 + STRIDE * TWIN,
                ],
                in_=y_t,
            )


def _prep_weights(weight: np.ndarray, variant=None) -> np.ndarray:
    import ml_dtypes

    variant = variant or VARIANT
    w2 = weight.reshape(CIN, K).astype(np.float32)
    p = np.arange(128)
    if variant in ("qp4", "qp4p"):
        wd = np.zeros((128, 64 * 64), np.float32)
        for q in range(4):
            for r in range(8):
                for tap in range(2):
                    blk = (q * 8 + r) * 2 + tap
                    # col = co_l, nonzero rows p = 2*co_l + j
                    wd[p, blk * 64 + p // 2] = w2[128 * q + p, r + 8 * tap]
    elif variant == "g2":
        wd = np.zeros((128, 32 * 128), np.float32)
        for q in range(4):
            for m in range(4):
                for tap in range(2):
                    blk = (q * 4 + m) * 2 + tap
                    for u in range(2):
                        # col = 64u + co_l, nonzero rows p = 2*co_l + j
                        wd[p, blk * 128 + 64 * u + p // 2] = w2[
                            128 * q + p, m + 4 * u + 8 * tap
                        ]
    else:
        wd = np.zeros((128, 64 * 128), np.float32)
        for ct in range(2):
            for j in range(2):
                for k in range(K):
                    base = ((ct * 2 + j) * 16 + k) * 128
                    wd[p, base + p] = w2[256 * ct + 2 * p + j, k]
    return wd.astype(ml_dtypes.bfloat16)


def _prep_bias(bias: np.ndarray, variant=None) -> np.ndarray:
    variant = variant or VARIANT
    p = np.arange(128)
    b4 = np.zeros((128, 4), np.float32)
    if variant in ("qp4", "qp4p"):
        for g in range(2):
            b4[:, g] = bias[128 * g + p]
    elif variant == "g2":
        for q in range(4):
            b4[:, q] = bias[64 * q + p % 64]
    else:
        for ct in range(2):
            b4[:, 2 * ct] = bias[128 * ct + p]
    return b4


def _make_exec(nc):
    """Build a jitted 8-core SPMD callable for a Bass module."""
    import jax
    import concourse.mybir as mybir
    from concourse import bass2jax
    from jax.sharding import Mesh, PartitionSpec
    from jax.experimental.shard_map import shard_map

    bass2jax.install_neuronx_cc_hook()

    partition_name = nc.partition_id_tensor.name if nc.partition_id_tensor else None

    in_names = []
    out_names = []
    out_avals = []
    zero_outs = []
    for alloc in nc.m.functions[0].allocations:
        if not isinstance(alloc, mybir.MemoryLocationSet):
            continue
        name = alloc.memorylocations[0].name
        if alloc.kind == "ExternalInput":
            if name != partition_name:
                in_names.append(name)
        elif alloc.kind == "ExternalOutput":
            shape = tuple(alloc.tensor_shape)
            dtype = mybir.dt.np(alloc.dtype)
            out_names.append(name)
            out_avals.append(jax.core.ShapedArray(shape, dtype))
            zero_outs.append(np.zeros(shape, dtype))
    n_params = len(in_names)
    all_in_names = list(in_names) + list(out_names)
    if partition_name is not None:
        all_in_names.append(partition_name)

    def _body(*args):
        operands = list(args)
        if partition_name is not None:
            operands.append(bass2jax.partition_id_tensor())
        outs = bass2jax._bass_exec_p.bind(
            *operands,
            out_avals=tuple(out_avals),
            in_names=tuple(all_in_names),
            out_names=tuple(out_names),
            lowering_input_output_aliases=(),
            sim_require_finite=True,
            sim_require_nnan=True,
            nc=nc,
        )
        return tuple(outs)

    devices = jax.devices()[:NCORES]
    mesh = Mesh(np.asarray(devices), ("core",))
    n_outs = len(out_names)
    in_specs = (PartitionSpec("core"),) * (n_params + n_outs)
    out_specs = (PartitionSpec("core"),) * n_outs
    sharded = jax.jit(
        shard_map(
            _body, mesh=mesh, in_specs=in_specs, out_specs=out_specs, check_rep=False
        ),
        keep_unused=True,
    )
    concat_zeros = [
        np.zeros((NCORES * z.shape[0], *z.shape[1:]), z.dtype) for z in zero_outs
    ]
    return (sharded, in_names, out_names, out_avals, concat_zeros)


def _get_exec():
    if "exec" not in _CACHE:
        nc = _build_nc()
        _CACHE["nc"] = nc
        _CACHE["exec"] = _make_exec(nc)
    return _CACHE["exec"]


def _make_concat_inputs(x, weight, bias):
    """Per-core input dict -> concatenated global arrays (order = in_names)."""
    import ml_dtypes

    wd = _prep_weights(weight)
    bias4 = _prep_bias(bias)
    xp = np.zeros((NCORES, CIN, 1 + T), ml_dtypes.bfloat16)
    xp[:, :, 1:] = x.astype(ml_dtypes.bfloat16)
    per_core = {
        "x": xp.reshape(NCORES * CIN, 1 + T),
        "wd": np.concatenate([wd] * NCORES, axis=0),
        "bias": np.concatenate([bias4] * NCORES, axis=0),
    }
    return per_core


def kernel(x, weight, bias) -> np.ndarray:
    x = np.asarray(x, dtype=np.float32)
    weight = np.asarray(weight, dtype=np.float32)
    bias = np.asarray(bias, dtype=np.float32)

    sharded, in_names, out_names, out_avals, concat_zeros = _get_exec()
    per_core = _make_concat_inputs(x, weight, bias)
    concat_in = [per_core[name] for name in in_names]
    out_arrs = sharded(*concat_in, *concat_zeros)
    yi = out_names.index("y")
    out = np.asarray(out_arrs[yi]).reshape(NCORES, COUT, SOUT)
    return out.astype(np.float32)


# revision 17
# speedup vs baseline: 1.3983x; 1.3983x over previous
"""Causal ConvTranspose1d (grouped, stride 8) Trainium2 Bass kernel.

Problem (hardcoded):
  x      [8, 512, 4096]  f32
  weight [512, 16, 1]    f32
  bias   [256]           f32
  out    [8, 256, 32768] f32   (= [B, Cout, T*stride])

Math (derived from the reference grouped dilated conv):
  with w2 = weight.reshape(512, 16), cpg = 2, stride = 8, K = 16:
  y[b, co, 8*t + r] = sum_{j in 0..1} ( w2[2co+j, r]   * x[b, 2co+j, t]
                                      + w2[2co+j, r+8] * x[b, 2co+j, t-1] )
                      + bias[co]          (x[., -1] == 0)

Sharding: data-parallel over batch; one batch element per NeuronCore (8 cores).

Variant "g2" (default):
  - x is loaded bf16 in natural 128-channel slices q = 0..3 ([128, 1+4096],
    leading zero column for the t-1 tap).
  - Pair-packed stationary matrices [128, 128] bf16 with 2 nonzeros per
    column: column 64*u + co_l accumulates both input channels (j = 0, 1)
    of output channel 64*q + co_l for phase r = m + 4*u.  One matmul per
    (m, tap) covers both j terms -> half the PE column streams of the
    diagonal formulation.
  - Matmuls write PSUM with free-dim stride 4 (offset m), so a PSUM bank
    [128 = (u, co_l), 512 = (t, m)] holds 128 t x 8 phases with phases
    m = 0..3 interleaved; the drain copies then write 16B-contiguous runs
    (4 f32) per output-time step instead of isolated 4B elements.
  - Drain copies (+ bias add) are split across Scalar, Vector and GpSimd.
  - [64, 4096] f32 staging tiles are DMAed to HBM contiguously.

Variant "h": diagonal stationaries (1 nonzero/col, 128-wide PSUM,
phase-major) as the previous baseline, but bf16 inputs and the drain
copies split across all three engines.
"""

import numpy as np

B, CIN, COUT, K, T = 8, 512, 256, 16, 4096
STRIDE = 8
SOUT = T * STRIDE  # 32768
NCORES = 8
VARIANT = "qp4"

_CACHE = {}


def _build_nc(variant=None):
    import concourse.mybir as mybir
    from concourse import bacc
    from concourse.tile import TileContext

    f32 = mybir.dt.float32
    bf16 = mybir.dt.bfloat16
    variant = variant or VARIANT

    nc = bacc.Bacc(trn_type="TRN2", target_bir_lowering=False, debug=False)

    if variant in ("qp4", "qp4p"):
        # 64 pair stationaries [128, 64]: blk = (q*8 + r)*2 + tap
        WDCOLS = 64 * 64
    elif variant == "g2":
        # 32 pair stationaries: blk = (q*4 + m)*2 + tap, each [128, 128]
        WDCOLS = 32 * 128
    else:
        # 64 diag stationaries: blk = ((ct*2 + j)*16 + k), each [128, 128]
        WDCOLS = 64 * 128
    x = nc.dram_tensor("x", [CIN, 1 + T], bf16, kind="ExternalInput").ap()
    wd = nc.dram_tensor("wd", [128, WDCOLS], bf16, kind="ExternalInput").ap()
    bias = nc.dram_tensor("bias", [128, 4], f32, kind="ExternalInput").ap()
    y = nc.dram_tensor("y", [COUT, SOUT], f32, kind="ExternalOutput").ap()

    # weighted round-robin over the two PSUM-capable copy engines
    # (GpSimd/Pool cannot access PSUM).  Act @1.2GHz vs DVE @0.96GHz
    # -> 5:4 split balances their busy time.
    def engines(nc):
        pat = [nc.scalar, nc.vector] * 4 + [nc.scalar]

        def pick(i):
            return pat[i % len(pat)]

        return pick

    with TileContext(nc) as tc:
        ps_bufs = 1 if variant in ("qp4", "qp4p") else 2
        y_bufs = 3 if variant in ("qp4", "qp4p") else 3
        with (
            tc.tile_pool(name="const", bufs=1) as cpool,
            tc.tile_pool(name="xp", bufs=2) as xpool,
            tc.tile_pool(name="yp", bufs=y_bufs) as ypool,
            tc.tile_pool(name="ps", bufs=ps_bufs, space="PSUM") as pspool,
        ):
            wd_t = cpool.tile([128, WDCOLS], bf16)
            nc.sync.dma_start(out=wd_t, in_=wd)
            bias_t = cpool.tile([128, 4], f32)
            nc.sync.dma_start(out=bias_t, in_=bias)
            pick = engines(nc)

            if variant in ("qp4", "qp4p"):
                _emit_qp4(
                    nc, tc, xpool, ypool, pspool, x, y, wd_t, bias_t, pick,
                    preload=(variant == "qp4p"),
                )
            elif variant == "g2":
                _emit_g2(nc, tc, xpool, ypool, pspool, x, y, wd_t, bias_t, pick)
            else:
                _emit_h(nc, tc, xpool, ypool, pspool, x, y, wd_t, bias_t, pick)
    nc.compile()
    return nc


def _emit_qp4(nc, tc, xpool, ypool, pspool, x, y, wd_t, bias_t, pick, preload=False):
    """q-pair col-tiled variant: PSUM partitions = 128 contiguous co.

    Two M=64 matmuls per (r, tap) — one per q-slice of the co-pair — at
    tile_position (0, 0) and (0, 64) fill a [128, 512] PSUM bank that
    holds 128 t x 4 phases (free = 4*tl + (r - 4h)).  Copies are 128-wide
    with 16B-contiguous runs; y staging spans 128 partitions so the DMA
    reads at full rate.
    """
    import concourse.mybir as mybir

    f32 = mybir.dt.float32
    bf16 = mybir.dt.bfloat16
    TCH = 128            # t per PSUM bank
    NSG = 8              # staging groups per co-pair
    SGT = T // NSG       # 1024 t per staging group
    NB = SGT // TCH      # 8 banks per (staging group, r-half)
    ci = 0
    for g in range(2):
        xg = []
        for gh in range(2):
            q = 2 * g + gh
            x_t = xpool.tile([128, 1 + T], bf16, tag=f"x{gh}", name=f"x_t{gh}")
            nc.sync.dma_start(out=x_t, in_=x[128 * q : 128 * (q + 1), :])
            xg.append(x_t)
        for sg in range(NSG):
            y_t = ypool.tile([128, STRIDE * SGT], f32, tag="y", name="y_t")
            y_r = y_t.rearrange("p (t e) -> p t e", e=STRIDE)
            # Bank-outer order: each bank finishes its 16 matmuls
            # consecutively so its drain copy overlaps the next bank's
            # matmuls instead of all 8 banks completing at once.
            for b in range(NB):
                t0 = (sg * NB + b) * TCH
                for h in range(2):
                    p_t = pspool.tile(
                        [128, 4 * TCH], f32,
                        tag=f"ps{b % 4}_{h}", name=f"p_t{b % 4}_{h}",
                    )
                    for rr in range(4):
                        r = 4 * h + rr
                        for tap in range(2):
                            mms = []
                            for gh in range(2):
                                blk = ((2 * g + gh) * 8 + r) * 2 + tap
                                lhsT = wd_t[:, blk * 64 : (blk + 1) * 64]
                                if preload:
                                    nc.tensor.ldweights(
                                        lhsT, tile_position=(0, 64 * gh)
                                    )
                            for gh in range(2):
                                blk = ((2 * g + gh) * 8 + r) * 2 + tap
                                lhsT = wd_t[:, blk * 64 : (blk + 1) * 64]
                                rhs = xg[gh][
                                    :, (1 - tap) + t0 : (1 - tap) + t0 + TCH
                                ]
                                mm = nc.tensor.matmul(
                                    p_t[64 * gh : 64 * (gh + 1), rr : 4 * TCH : 4],
                                    lhsT,
                                    rhs,
                                    start=(tap == 0),
                                    stop=(tap == 1),
                                    tile_position=(0, 64 * gh),
                                )
                                if preload:
                                    mm.ins.ldweights = False
                    dst = y_r[:, TCH * b : TCH * (b + 1), 4 * h : 4 * h + 4]
                    b_ap = bias_t[:, g : g + 1]
                    eng = pick(ci)
                    ci += 1
                    if eng is nc.scalar:
                        eng.add(dst, p_t, b_ap)
                    else:
                        eng.tensor_scalar_add(dst, p_t, b_ap)
            nc.sync.dma_start(
                out=y[
                    128 * g : 128 * (g + 1),
                    STRIDE * SGT * sg : STRIDE * SGT * (sg + 1),
                ],
                in_=y_t,
            )


def _emit_g2(nc, tc, xpool, ypool, pspool, x, y, wd_t, bias_t, pick):
    import concourse.mybir as mybir

    f32 = mybir.dt.float32
    bf16 = mybir.dt.bfloat16
    TCH = 128            # t per PSUM bank
    NB = T // TCH        # 32 banks' worth of t-chunks per q
    BPS = 4              # banks per staging tile ([64, 4096] = 512 t)
    ci = 0
    for q in range(4):
        x_t = xpool.tile([128, 1 + T], bf16, tag="x", name="x_t")
        nc.sync.dma_start(out=x_t, in_=x[128 * q : 128 * (q + 1), :])
        for sg in range(NB // BPS):  # staging groups of 4 banks
            y_t = ypool.tile([64, STRIDE * TCH * BPS], f32, tag="y", name="y_t")
            p_ts = []
            for b in range(BPS):
                t0 = (sg * BPS + b) * TCH
                p_t = pspool.tile([128, 4 * TCH], f32, tag=f"ps{b}", name=f"p_t{b}")
                p_ts.append(p_t)
                for m in range(4):
                    blk = (q * 4 + m) * 2
                    for tap in range(2):
                        rhs = x_t[:, (1 - tap) + t0 : (1 - tap) + t0 + TCH]
                        nc.tensor.matmul(
                            p_t[:, m : 4 * TCH : 4],
                            wd_t[:, (blk + tap) * 128 : (blk + tap + 1) * 128],
                            rhs,
                            start=(tap == 0),
                            stop=(tap == 1),
                        )
            y_r = y_t.rearrange("p (t e) -> p t e", e=STRIDE)
            for b in range(BPS):
                for u in range(2):
                    # P[64u + co_l, 4*tl + m] -> y_t[co_l, 8*tl + 4u + m]
                    dst = y_r[:, TCH * b : TCH * (b + 1), 4 * u : 4 * u + 4]
                    src = p_ts[b][64 * u : 64 * (u + 1), :]
                    b_ap = bias_t[64 * u : 64 * (u + 1), q : q + 1]
                    eng = pick(ci)
                    ci += 1
                    if eng is nc.scalar:
                        eng.add(dst, src, b_ap)
                    else:
                        eng.tensor_scalar_add(dst, src, b_ap)
            nc.sync.dma_start(
                out=y[
                    64 * q : 64 * (q + 1),
                    STRIDE * TCH * BPS * sg : STRIDE * TCH * BPS * (sg + 1),
                ],
                in_=y_t,
            )


def _emit_h(nc, tc, xpool, ypool, pspool, x, y, wd_t, bias_t, pick):
    import concourse.mybir as mybir

    f32 = mybir.dt.float32
    bf16 = mybir.dt.bfloat16
    TWIN = 512
    NTWIN = T // TWIN  # 8
    ci = 0
    for ct in range(2):
        xj = []
        for j in range(2):
            x_t = xpool.tile([128, 1 + T], bf16, tag=f"x{j}", name=f"x_t{j}")
            src = x[256 * ct + j : 256 * ct + 256 : 2, :]
            nc.sync.dma_start(out=x_t, in_=src)
            xj.append(x_t)
        for twin in range(NTWIN):
            t0 = twin * TWIN
            y_t = ypool.tile([128, STRIDE * TWIN], f32, tag="y", name="y_t")
            for r in range(8):
                p_t = pspool.tile(
                    [128, TWIN], f32, tag=f"ps{r % 4}", name=f"p_t{r % 4}"
                )
                for j in range(2):
                    for tap in range(2):
                        k = r + 8 * tap
                        rhs = xj[j][:, (1 - tap) + t0 : (1 - tap) + t0 + TWIN]
                        col = ((ct * 2 + j) * 16 + k) * 128
                        nc.tensor.matmul(
                            p_t,
                            wd_t[:, col : col + 128],
                            rhs,
                            start=(tap == 0 and j == 0),
                            stop=(tap == 1 and j == 1),
                        )
                out_ap = y_t[:, r : STRIDE * TWIN : STRIDE]
                b_ap = bias_t[:, 2 * ct : 2 * ct + 1]
                eng = pick(ci)
                ci += 1
                if eng is nc.scalar:
                    eng.add(out_ap, p_t, b_ap)
                else:
                    eng.tensor_scalar_add(out_ap, p_t, b_ap)
            nc.sync.dma_start(
                out=y[
                    128 * ct : 128 * (ct + 1),
                    STRIDE * t0 : STRIDE * t0 + STRIDE * TWIN,
                ],
                in_=y_t,
            )


def _prep_weights(weight: np.ndarray, variant=None) -> np.ndarray:
    import ml_dtypes

    variant = variant or VARIANT
    w2 = weight.reshape(CIN, K).astype(np.float32)
    p = np.arange(128)
    if variant in ("qp4", "qp4p"):
        wd = np.zeros((128, 64 * 64), np.float32)
        for q in range(4):
            for r in range(8):
                for tap in range(2):
                    blk = (q * 8 + r) * 2 + tap
                    # col = co_l, nonzero rows p = 2*co_l + j
                    wd[p, blk * 64 + p // 2] = w2[128 * q + p, r + 8 * tap]
    elif variant == "g2":
        wd = np.zeros((128, 32 * 128), np.float32)
        for q in range(4):
            for m in range(4):
                for tap in range(2):
                    blk = (q * 4 + m) * 2 + tap
                    for u in range(2):
                        # col = 64u + co_l, nonzero rows p = 2*co_l + j
                        wd[p, blk * 128 + 64 * u + p // 2] = w2[
                            128 * q + p, m + 4 * u + 8 * tap
                        ]
    else:
        wd = np.zeros((128, 64 * 128), np.float32)
        for ct in range(2):
            for j in range(2):
                for k in range(K):
                    base = ((ct * 2 + j) * 16 + k) * 128
                    wd[p, base + p] = w2[256 * ct + 2 * p + j, k]
    return wd.astype(ml_dtypes.bfloat16)


def _prep_bias(bias: np.ndarray, variant=None) -> np.ndarray:
    variant = variant or VARIANT
    p = np.arange(128)
    b4 = np.zeros((128, 4), np.float32)
    if variant in ("qp4", "qp4p"):
        for g in range(2):
            b4[:, g] = bias[128 * g + p]
    elif variant == "g2":
        for q in range(4):
            b4[:, q] = bias[64 * q + p % 64]
    else:
        for ct in range(2):
            b4[:, 2 * ct] = bias[128 * ct + p]
    return b4


def _make_exec(nc):
    """Build a jitted 8-core SPMD callable for a Bass module."""
    import jax
    import concourse.mybir as mybir
    from concourse import bass2jax
    from jax.sharding import Mesh, PartitionSpec
    from jax.experimental.shard_map import shard_map

    bass2jax.install_neuronx_cc_hook()

    partition_name = nc.partition_id_tensor.name if nc.partition_id_tensor else None

    in_names = []
    out_names = []
    out_avals = []
    zero_outs = []
    for alloc in nc.m.functions[0].allocations:
        if not isinstance(alloc, mybir.MemoryLocationSet):
            continue
        name = alloc.memorylocations[0].name
        if alloc.kind == "ExternalInput":
            if name != partition_name:
                in_names.append(name)
        elif alloc.kind == "ExternalOutput":
            shape = tuple(alloc.tensor_shape)
            dtype = mybir.dt.np(alloc.dtype)
            out_names.append(name)
            out_avals.append(jax.core.ShapedArray(shape, dtype))
            zero_outs.append(np.zeros(shape, dtype))
    n_params = len(in_names)
    all_in_names = list(in_names) + list(out_names)
    if partition_name is not None:
        all_in_names.append(partition_name)

    def _body(*args):
        operands = list(args)
        if partition_name is not None:
            operands.append(bass2jax.partition_id_tensor())
        outs = bass2jax._bass_exec_p.bind(
            *operands,
            out_avals=tuple(out_avals),
            in_names=tuple(all_in_names),
            out_names=tuple(out_names),
            lowering_input_output_aliases=(),
            sim_require_finite=True,
            sim_require_nnan=True,
            nc=nc,
        )
        return tuple(outs)

    devices = jax.devices()[:NCORES]
    mesh = Mesh(np.asarray(devices), ("core",))
    n_outs = len(out_names)
    in_specs = (PartitionSpec("core"),) * (n_params + n_outs)
    out_specs = (PartitionSpec("core"),) * n_outs
    sharded = jax.jit(
        shard_map(
            _body, mesh=mesh, in_specs=in_specs, out_specs=out_specs, check_rep=False
        ),
        keep_unused=True,
    )
    concat_zeros = [
        np.zeros((NCORES * z.shape[0], *z.shape[1:]), z.dtype) for z in zero_outs
    ]
    return (sharded, in_names, out_names, out_avals, concat_zeros)


def _get_exec():
    if "exec" not in _CACHE:
        nc = _build_nc()
        _CACHE["nc"] = nc
        _CACHE["exec"] = _make_exec(nc)
    return _CACHE["exec"]


def _make_concat_inputs(x, weight, bias):
    """Per-core input dict -> concatenated global arrays (order = in_names)."""
    import ml_dtypes

    wd = _prep_weights(weight)
    bias4 = _prep_bias(bias)
    xp = np.zeros((NCORES, CIN, 1 + T), ml_dtypes.bfloat16)
    xp[:, :, 1:] = x.astype(ml_dtypes.bfloat16)
    per_core = {
        "x": xp.reshape(NCORES * CIN, 1 + T),
        "wd": np.concatenate([wd] * NCORES, axis=0),
        "bias": np.concatenate([bias4] * NCORES, axis=0),
    }
    return per_core


def kernel(x, weight, bias) -> np.ndarray:
    x = np.asarray(x, dtype=np.float32)
    weight = np.asarray(weight, dtype=np.float32)
    bias = np.asarray(bias, dtype=np.float32)

    sharded, in_names, out_names, out_avals, concat_zeros = _get_exec()
    per_core = _make_concat_inputs(x, weight, bias)
    concat_in = [per_core[name] for name in in_names]
    out_arrs = sharded(*concat_in, *concat_zeros)
    yi = out_names.index("y")
    out = np.asarray(out_arrs[yi]).reshape(NCORES, COUT, SOUT)
    return out.astype(np.float32)


# revision 20
# speedup vs baseline: 1.4290x; 1.0219x over previous
"""Causal ConvTranspose1d (grouped, stride 8) Trainium2 Bass kernel.

Problem (hardcoded):
  x      [8, 512, 4096]  f32
  weight [512, 16, 1]    f32
  bias   [256]           f32
  out    [8, 256, 32768] f32   (= [B, Cout, T*stride])

Math (derived from the reference grouped dilated conv):
  with w2 = weight.reshape(512, 16), cpg = 2, stride = 8, K = 16:
  y[b, co, 8*t + r] = sum_{j in 0..1} ( w2[2co+j, r]   * x[b, 2co+j, t]
                                      + w2[2co+j, r+8] * x[b, 2co+j, t-1] )
                      + bias[co]          (x[., -1] == 0)

Sharding: data-parallel over batch; one batch element per NeuronCore (8 cores).

Variant "g2" (default):
  - x is loaded bf16 in natural 128-channel slices q = 0..3 ([128, 1+4096],
    leading zero column for the t-1 tap).
  - Pair-packed stationary matrices [128, 128] bf16 with 2 nonzeros per
    column: column 64*u + co_l accumulates both input channels (j = 0, 1)
    of output channel 64*q + co_l for phase r = m + 4*u.  One matmul per
    (m, tap) covers both j terms -> half the PE column streams of the
    diagonal formulation.
  - Matmuls write PSUM with free-dim stride 4 (offset m), so a PSUM bank
    [128 = (u, co_l), 512 = (t, m)] holds 128 t x 8 phases with phases
    m = 0..3 interleaved; the drain copies then write 16B-contiguous runs
    (4 f32) per output-time step instead of isolated 4B elements.
  - Drain copies (+ bias add) are split across Scalar, Vector and GpSimd.
  - [64, 4096] f32 staging tiles are DMAed to HBM contiguously.

Variant "h": diagonal stationaries (1 nonzero/col, 128-wide PSUM,
phase-major) as the previous baseline, but bf16 inputs and the drain
copies split across all three engines.
"""

import numpy as np

B, CIN, COUT, K, T = 8, 512, 256, 16, 4096
STRIDE = 8
SOUT = T * STRIDE  # 32768
NCORES = 8
VARIANT = "qp4"

_CACHE = {}


def _build_nc(variant=None):
    import concourse.mybir as mybir
    from concourse import bacc
    from concourse.tile import TileContext

    f32 = mybir.dt.float32
    bf16 = mybir.dt.bfloat16
    variant = variant or VARIANT

    nc = bacc.Bacc(trn_type="TRN2", target_bir_lowering=False, debug=False)

    if variant in ("qp4", "qp4p"):
        # 64 pair stationaries [128, 64]: blk = (q*8 + r)*2 + tap
        WDCOLS = 64 * 64
    elif variant == "g2":
        # 32 pair stationaries: blk = (q*4 + m)*2 + tap, each [128, 128]
        WDCOLS = 32 * 128
    else:
        # 64 diag stationaries: blk = ((ct*2 + j)*16 + k), each [128, 128]
        WDCOLS = 64 * 128
    x = nc.dram_tensor("x", [CIN, 1 + T], bf16, kind="ExternalInput").ap()
    wd = nc.dram_tensor("wd", [128, WDCOLS], bf16, kind="ExternalInput").ap()
    bias = nc.dram_tensor("bias", [128, 4], f32, kind="ExternalInput").ap()
    y = nc.dram_tensor("y", [COUT, SOUT], f32, kind="ExternalOutput").ap()

    # weighted round-robin over the two PSUM-capable copy engines
    # (GpSimd/Pool cannot access PSUM).  Act @1.2GHz vs DVE @0.96GHz
    # -> 5:4 split balances their busy time.
    def engines(nc):
        pat = [nc.scalar, nc.vector] * 4 + [nc.scalar]

        def pick(i):
            return pat[i % len(pat)]

        return pick

    with TileContext(nc) as tc:
        ps_bufs = 1 if variant in ("qp4", "qp4p") else 2
        y_bufs = 3 if variant in ("qp4", "qp4p") else 3
        with (
            tc.tile_pool(name="const", bufs=1) as cpool,
            tc.tile_pool(name="xp", bufs=2) as xpool,
            tc.tile_pool(name="yp", bufs=y_bufs) as ypool,
            tc.tile_pool(name="ps", bufs=ps_bufs, space="PSUM") as pspool,
        ):
            bias_t = cpool.tile([128, 4], f32)
            nc.sync.dma_start(out=bias_t, in_=bias)
            pick = engines(nc)

            if variant in ("qp4", "qp4p"):
                # split wd per co-pair so the first matmul only waits on half
                wd_ts = []
                for g in range(2):
                    wd_g = cpool.tile([128, WDCOLS // 2], bf16, name=f"wd{g}")
                    nc.sync.dma_start(
                        out=wd_g, in_=wd[:, g * (WDCOLS // 2) : (g + 1) * (WDCOLS // 2)]
                    )
                    wd_ts.append(wd_g)
                _emit_qp4(
                    nc, tc, xpool, ypool, pspool, x, y, wd_ts, bias_t, pick,
                    preload=(variant == "qp4p"),
                )
            else:
                wd_t = cpool.tile([128, WDCOLS], bf16)
                nc.sync.dma_start(out=wd_t, in_=wd)
                if variant == "g2":
                    _emit_g2(nc, tc, xpool, ypool, pspool, x, y, wd_t, bias_t, pick)
                else:
                    _emit_h(nc, tc, xpool, ypool, pspool, x, y, wd_t, bias_t, pick)
    nc.compile()
    return nc


def _emit_qp4(nc, tc, xpool, ypool, pspool, x, y, wd_ts, bias_t, pick, preload=False):
    """q-pair col-tiled variant: PSUM partitions = 128 contiguous co.

    Two M=64 matmuls per (r, tap) — one per q-slice of the co-pair — at
    tile_position (0, 0) and (0, 64) fill a [128, 512] PSUM bank that
    holds 128 t x 4 phases (free = 4*tl + (r - 4h)).  Copies are 128-wide
    with 16B-contiguous runs; y staging spans 128 partitions so the DMA
    reads at full rate.  x is loaded in 1024-t chunks (with a 1-column
    overlap for the t-1 tap) so the first matmuls start after ~1 MB of
    input DMA instead of the full x.
    """
    import concourse.mybir as mybir

    f32 = mybir.dt.float32
    bf16 = mybir.dt.bfloat16
    TCH = 128            # t per PSUM bank
    NSG = 16             # staging groups per co-pair
    SGT = T // NSG       # 256 t per staging group
    NB = SGT // TCH      # 2 banks per (staging group, r-half)
    XC = 1024            # t per x chunk
    ci = 0
    for g in range(2):
        wd_g = wd_ts[g]
        cur = [None, None]
        curq = -1
        for sg in range(NSG):
            qh = (sg * SGT) // XC
            if qh != curq:
                for gh in range(2):
                    q = 2 * g + gh
                    x_t = xpool.tile(
                        [128, 1 + XC], bf16,
                        tag=f"x{gh}{qh % 2}", name=f"x_t{gh}{qh % 2}",
                    )
                    nc.sync.dma_start(
                        out=x_t,
                        in_=x[128 * q : 128 * (q + 1), XC * qh : XC * qh + 1 + XC],
                    )
                    cur[gh] = x_t
                curq = qh
            y_t = ypool.tile([128, STRIDE * SGT], f32, tag="y", name="y_t")
            y_r = y_t.rearrange("p (t e) -> p t e", e=STRIDE)
            # Bank-outer order: each bank finishes its 16 matmuls
            # consecutively so its drain copy overlaps the next bank's
            # matmuls instead of all banks completing at once.
            for b in range(NB):
                tl = (sg * SGT + b * TCH) - XC * qh  # chunk-local t offset
                for h in range(2):
                    p_t = pspool.tile(
                        [128, 4 * TCH], f32,
                        tag=f"ps{b % 4}_{h}", name=f"p_t{b % 4}_{h}",
                    )
                    for rr in range(4):
                        r = 4 * h + rr
                        for tap in range(2):
                            for gh in range(2):
                                blk = (gh * 8 + r) * 2 + tap
                                lhsT = wd_g[:, blk * 64 : (blk + 1) * 64]
                                if preload:
                                    nc.tensor.ldweights(
                                        lhsT, tile_position=(0, 64 * gh)
                                    )
                                rhs = cur[gh][
                                    :, (1 - tap) + tl : (1 - tap) + tl + TCH
                                ]
                                mm = nc.tensor.matmul(
                                    p_t[64 * gh : 64 * (gh + 1), rr : 4 * TCH : 4],
                                    lhsT,
                                    rhs,
                                    start=(tap == 0),
                                    stop=(tap == 1),
                                    tile_position=(0, 64 * gh),
                                )
                                if preload:
                                    mm.ins.ldweights = False
                    dst = y_r[:, TCH * b : TCH * (b + 1), 4 * h : 4 * h + 4]
                    b_ap = bias_t[:, g : g + 1]
                    eng = pick(ci)
                    ci += 1
                    if eng is nc.scalar:
                        eng.add(dst, p_t, b_ap)
                    else:
                        eng.tensor_scalar_add(dst, p_t, b_ap)
            nc.sync.dma_start(
                out=y[
                    128 * g : 128 * (g + 1),
                    STRIDE * SGT * sg : STRIDE * SGT * (sg + 1),
                ],
                in_=y_t,
            )


def _emit_g2(nc, tc, xpool, ypool, pspool, x, y, wd_t, bias_t, pick):
    import concourse.mybir as mybir

    f32 = mybir.dt.float32
    bf16 = mybir.dt.bfloat16
    TCH = 128            # t per PSUM bank
    NB = T // TCH        # 32 banks' worth of t-chunks per q
    BPS = 4              # banks per staging tile ([64, 4096] = 512 t)
    ci = 0
    for q in range(4):
        x_t = xpool.tile([128, 1 + T], bf16, tag="x", name="x_t")
        nc.sync.dma_start(out=x_t, in_=x[128 * q : 128 * (q + 1), :])
        for sg in range(NB // BPS):  # staging groups of 4 banks
            y_t = ypool.tile([64, STRIDE * TCH * BPS], f32, tag="y", name="y_t")
            p_ts = []
            for b in range(BPS):
                t0 = (sg * BPS + b) * TCH
                p_t = pspool.tile([128, 4 * TCH], f32, tag=f"ps{b}", name=f"p_t{b}")
                p_ts.append(p_t)
                for m in range(4):
                    blk = (q * 4 + m) * 2
                    for tap in range(2):
                        rhs = x_t[:, (1 - tap) + t0 : (1 - tap) + t0 + TCH]
                        nc.tensor.matmul(
                            p_t[:, m : 4 * TCH : 4],
                            wd_t[:, (blk + tap) * 128 : (blk + tap + 1) * 128],
                            rhs,
                            start=(tap == 0),
                            stop=(tap == 1),
                        )
            y_r = y_t.rearrange("p (t e) -> p t e", e=STRIDE)
            for b in range(BPS):
                for u in range(2):
                    # P[64u + co_l, 4*tl + m] -> y_t[co_l, 8*tl + 4u + m]
                    dst = y_r[:, TCH * b : TCH * (b + 1), 4 * u : 4 * u + 4]
                    src = p_ts[b][64 * u : 64 * (u + 1), :]
                    b_ap = bias_t[64 * u : 64 * (u + 1), q : q + 1]
                    eng = pick(ci)
                    ci += 1
                    if eng is nc.scalar:
                        eng.add(dst, src, b_ap)
                    else:
                        eng.tensor_scalar_add(dst, src, b_ap)
            nc.sync.dma_start(
                out=y[
                    64 * q : 64 * (q + 1),
                    STRIDE * TCH * BPS * sg : STRIDE * TCH * BPS * (sg + 1),
                ],
                in_=y_t,
            )


def _emit_h(nc, tc, xpool, ypool, pspool, x, y, wd_t, bias_t, pick):
    import concourse.mybir as mybir

    f32 = mybir.dt.float32
    bf16 = mybir.dt.bfloat16
    TWIN = 512
    NTWIN = T // TWIN  # 8
    ci = 0
    for ct in range(2):
        xj = []
        for j in range(2):
            x_t = xpool.tile([128, 1 + T], bf16, tag=f"x{j}", name=f"x_t{j}")
            src = x[256 * ct + j : 256 * ct + 256 : 2, :]
            nc.sync.dma_start(out=x_t, in_=src)
            xj.append(x_t)
        for twin in range(NTWIN):
            t0 = twin * TWIN
            y_t = ypool.tile([128, STRIDE * TWIN], f32, tag="y", name="y_t")
            for r in range(8):
                p_t = pspool.tile(
                    [128, TWIN], f32, tag=f"ps{r % 4}", name=f"p_t{r % 4}"
                )
                for j in range(2):
                    for tap in range(2):
                        k = r + 8 * tap
                        rhs = xj[j][:, (1 - tap) + t0 : (1 - tap) + t0 + TWIN]
                        col = ((ct * 2 + j) * 16 + k) * 128
                        nc.tensor.matmul(
                            p_t,
                            wd_t[:, col : col + 128],
                            rhs,
                            start=(tap == 0 and j == 0),
                            stop=(tap == 1 and j == 1),
                        )
                out_ap = y_t[:, r : STRIDE * TWIN : STRIDE]
                b_ap = bias_t[:, 2 * ct : 2 * ct + 1]
                eng = pick(ci)
                ci += 1
                if eng is nc.scalar:
                    eng.add(out_ap, p_t, b_ap)
                else:
                    eng.tensor_scalar_add(out_ap, p_t, b_ap)
            nc.sync.dma_start(
                out=y[
                    128 * ct : 128 * (ct + 1),
                    STRIDE * t0 : STRIDE * t0 + STRIDE * TWIN,
                ],
                in_=y_t,
            )


def _prep_weights(weight: np.ndarray, variant=None) -> np.ndarray:
    import ml_dtypes

    variant = variant or VARIANT
    w2 = weight.reshape(CIN, K).astype(np.float32)
    p = np.arange(128)
    if variant in ("qp4", "qp4p"):
        wd = np.zeros((128, 64 * 64), np.float32)
        for q in range(4):
            for r in range(8):
                for tap in range(2):
                    blk = (q * 8 + r) * 2 + tap
                    # col = co_l, nonzero rows p = 2*co_l + j
                    wd[p, blk * 64 + p // 2] = w2[128 * q + p, r + 8 * tap]
    elif variant == "g2":
        wd = np.zeros((128, 32 * 128), np.float32)
        for q in range(4):
            for m in range(4):
                for tap in range(2):
                    blk = (q * 4 + m) * 2 + tap
                    for u in range(2):
                        # col = 64u + co_l, nonzero rows p = 2*co_l + j
                        wd[p, blk * 128 + 64 * u + p // 2] = w2[
                            128 * q + p, m + 4 * u + 8 * tap
                        ]
    else:
        wd = np.zeros((128, 64 * 128), np.float32)
        for ct in range(2):
            for j in range(2):
                for k in range(K):
                    base = ((ct * 2 + j) * 16 + k) * 128
                    wd[p, base + p] = w2[256 * ct + 2 * p + j, k]
    return wd.astype(ml_dtypes.bfloat16)


def _prep_bias(bias: np.ndarray, variant=None) -> np.ndarray:
    variant = variant or VARIANT
    p = np.arange(128)
    b4 = np.zeros((128, 4), np.float32)
    if variant in ("qp4", "qp4p"):
        for g in range(2):
            b4[:, g] = bias[128 * g + p]
    elif variant == "g2":
        for q in range(4):
            b4[:, q] = bias[64 * q + p % 64]
    else:
        for ct in range(2):
            b4[:, 2 * ct] = bias[128 * ct + p]
    return b4


def _make_exec(nc):
    """Build a jitted 8-core SPMD callable for a Bass module."""
    import jax
    import concourse.mybir as mybir
    from concourse import bass2jax
    from jax.sharding import Mesh, PartitionSpec
    from jax.experimental.shard_map import shard_map

    bass2jax.install_neuronx_cc_hook()

    partition_name = nc.partition_id_tensor.name if nc.partition_id_tensor else None

    in_names = []
    out_names = []
    out_avals = []
    zero_outs = []
    for alloc in nc.m.functions[0].allocations:
        if not isinstance(alloc, mybir.MemoryLocationSet):
            continue
        name = alloc.memorylocations[0].name
        if alloc.kind == "ExternalInput":
            if name != partition_name:
                in_names.append(name)
        elif alloc.kind == "ExternalOutput":
            shape = tuple(alloc.tensor_shape)
            dtype = mybir.dt.np(alloc.dtype)
            out_names.append(name)
            out_avals.append(jax.core.ShapedArray(shape, dtype))
            zero_outs.append(np.zeros(shape, dtype))
    n_params = len(in_names)
    all_in_names = list(in_names) + list(out_names)
    if partition_name is not None:
        all_in_names.append(partition_name)

    def _body(*args):
        operands = list(args)
        if partition_name is not None:
            operands.append(bass2jax.partition_id_tensor())
        outs = bass2jax._bass_exec_p.bind(
            *operands,
            out_avals=tuple(out_avals),
            in_names=tuple(all_in_names),
            out_names=tuple(out_names),
            lowering_input_output_aliases=(),
            sim_require_finite=True,
            sim_require_nnan=True,
            nc=nc,
        )
        return tuple(outs)

    devices = jax.devices()[:NCORES]
    mesh = Mesh(np.asarray(devices), ("core",))
    n_outs = len(out_names)
    in_specs = (PartitionSpec("core"),) * (n_params + n_outs)
    out_specs = (PartitionSpec("core"),) * n_outs
    sharded = jax.jit(
        shard_map(
            _body, mesh=mesh, in_specs=in_specs, out_specs=out_specs, check_rep=False
        ),
        keep_unused=True,
    )
    concat_zeros = [
        np.zeros((NCORES * z.shape[0], *z.shape[1:]), z.dtype) for z in zero_outs
    ]
    return (sharded, in_names, out_names, out_avals, concat_zeros)


def _get_exec():
    if "exec" not in _CACHE:
        nc = _build_nc()
        _CACHE["nc"] = nc
        _CACHE["exec"] = _make_exec(nc)
    return _CACHE["exec"]


def _make_concat_inputs(x, weight, bias):
    """Per-core input dict -> concatenated global arrays (order = in_names)."""
    import ml_dtypes

    wd = _prep_weights(weight)
    bias4 = _prep_bias(bias)
    xp = np.zeros((NCORES, CIN, 1 + T), ml_dtypes.bfloat16)
    xp[:, :, 1:] = x.astype(ml_dtypes.bfloat16)
    per_core = {
        "x": xp.reshape(NCORES * CIN, 1 + T),
        "wd": np.concatenate([wd] * NCORES, axis=0),
        "bias": np.concatenate([bias4] * NCORES, axis=0),
    }
    return per_core


def kernel(x, weight, bias) -> np.ndarray:
    x = np.asarray(x, dtype=np.float32)
    weight = np.asarray(weight, dtype=np.float32)
    bias = np.asarray(bias, dtype=np.float32)

    sharded, in_names, out_names, out_avals, concat_zeros = _get_exec()
    per_core = _make_concat_inputs(x, weight, bias)
    concat_in = [per_core[name] for name in in_names]
    out_arrs = sharded(*concat_in, *concat_zeros)
    yi = out_names.index("y")
    out = np.asarray(out_arrs[yi]).reshape(NCORES, COUT, SOUT)
    return out.astype(np.float32)


# revision 23
# speedup vs baseline: 1.4401x; 1.0078x over previous
"""Causal ConvTranspose1d (grouped, stride 8) Trainium2 Bass kernel.

Problem (hardcoded):
  x      [8, 512, 4096]  f32
  weight [512, 16, 1]    f32
  bias   [256]           f32
  out    [8, 256, 32768] f32   (= [B, Cout, T*stride])

Math (derived from the reference grouped dilated conv):
  with w2 = weight.reshape(512, 16), cpg = 2, stride = 8, K = 16:
  y[b, co, 8*t + r] = sum_{j in 0..1} ( w2[2co+j, r]   * x[b, 2co+j, t]
                                      + w2[2co+j, r+8] * x[b, 2co+j, t-1] )
                      + bias[co]          (x[., -1] == 0)

Sharding: data-parallel over batch; one batch element per NeuronCore (8 cores).

Variant "g2" (default):
  - x is loaded bf16 in natural 128-channel slices q = 0..3 ([128, 1+4096],
    leading zero column for the t-1 tap).
  - Pair-packed stationary matrices [128, 128] bf16 with 2 nonzeros per
    column: column 64*u + co_l accumulates both input channels (j = 0, 1)
    of output channel 64*q + co_l for phase r = m + 4*u.  One matmul per
    (m, tap) covers both j terms -> half the PE column streams of the
    diagonal formulation.
  - Matmuls write PSUM with free-dim stride 4 (offset m), so a PSUM bank
    [128 = (u, co_l), 512 = (t, m)] holds 128 t x 8 phases with phases
    m = 0..3 interleaved; the drain copies then write 16B-contiguous runs
    (4 f32) per output-time step instead of isolated 4B elements.
  - Drain copies (+ bias add) are split across Scalar, Vector and GpSimd.
  - [64, 4096] f32 staging tiles are DMAed to HBM contiguously.

Variant "h": diagonal stationaries (1 nonzero/col, 128-wide PSUM,
phase-major) as the previous baseline, but bf16 inputs and the drain
copies split across all three engines.
"""

import numpy as np

B, CIN, COUT, K, T = 8, 512, 256, 16, 4096
STRIDE = 8
SOUT = T * STRIDE  # 32768
NCORES = 8
VARIANT = "qp4"

_CACHE = {}


def _build_nc(variant=None):
    import concourse.mybir as mybir
    from concourse import bacc
    from concourse.tile import TileContext

    f32 = mybir.dt.float32
    bf16 = mybir.dt.bfloat16
    variant = variant or VARIANT

    nc = bacc.Bacc(trn_type="TRN2", target_bir_lowering=False, debug=False)

    if variant in ("qp4", "qp4p"):
        # 64 pair stationaries [128, 64]: blk = (q*8 + r)*2 + tap
        WDCOLS = 64 * 64
    elif variant == "g2":
        # 32 pair stationaries: blk = (q*4 + m)*2 + tap, each [128, 128]
        WDCOLS = 32 * 128
    else:
        # 64 diag stationaries: blk = ((ct*2 + j)*16 + k), each [128, 128]
        WDCOLS = 64 * 128
    x = nc.dram_tensor("x", [CIN, 1 + T], bf16, kind="ExternalInput").ap()
    wd = nc.dram_tensor("wd", [128, WDCOLS], bf16, kind="ExternalInput").ap()
    bias = nc.dram_tensor("bias", [128, 4], f32, kind="ExternalInput").ap()
    y = nc.dram_tensor("y", [COUT, SOUT], f32, kind="ExternalOutput").ap()

    # weighted round-robin over the two PSUM-capable copy engines
    # (GpSimd/Pool cannot access PSUM).  Act @1.2GHz vs DVE @0.96GHz
    # -> 5:4 split balances their busy time.
    def engines(nc):
        pat = [nc.scalar, nc.vector] * 4 + [nc.scalar]

        def pick(i):
            return pat[i % len(pat)]

        return pick

    with TileContext(nc) as tc:
        ps_bufs = 1 if variant in ("qp4", "qp4p") else 2
        y_bufs = 3 if variant in ("qp4", "qp4p") else 3
        with (
            tc.tile_pool(name="const", bufs=1) as cpool,
            tc.tile_pool(name="xp", bufs=2) as xpool,
            tc.tile_pool(name="yp", bufs=y_bufs) as ypool,
            tc.tile_pool(name="ps", bufs=ps_bufs, space="PSUM") as pspool,
        ):
            bias_t = cpool.tile([128, 4], f32)
            nc.sync.dma_start(out=bias_t, in_=bias)
            pick = engines(nc)

            if variant in ("qp4", "qp4p"):
                # split wd per co-pair so the first matmul only waits on half
                wd_ts = []
                for g in range(2):
                    wd_g = cpool.tile([128, WDCOLS // 2], bf16, name=f"wd{g}")
                    nc.sync.dma_start(
                        out=wd_g, in_=wd[:, g * (WDCOLS // 2) : (g + 1) * (WDCOLS // 2)]
                    )
                    wd_ts.append(wd_g)
                _emit_qp4(
                    nc, tc, xpool, ypool, pspool, x, y, wd_ts, bias_t, pick,
                    preload=(variant == "qp4p"),
                )
            else:
                wd_t = cpool.tile([128, WDCOLS], bf16)
                nc.sync.dma_start(out=wd_t, in_=wd)
                if variant == "g2":
                    _emit_g2(nc, tc, xpool, ypool, pspool, x, y, wd_t, bias_t, pick)
                else:
                    _emit_h(nc, tc, xpool, ypool, pspool, x, y, wd_t, bias_t, pick)
    nc.compile()
    return nc


def _emit_qp4(nc, tc, xpool, ypool, pspool, x, y, wd_ts, bias_t, pick, preload=False):
    """q-pair col-tiled variant: PSUM partitions = 128 contiguous co.

    Two M=64 matmuls per (r, tap) — one per q-slice of the co-pair — at
    tile_position (0, 0) and (0, 64) fill a [128, 512] PSUM bank that
    holds 128 t x 4 phases (free = 4*tl + (r - 4h)).  Copies are 128-wide
    with 16B-contiguous runs; y staging spans 128 partitions so the DMA
    reads at full rate.  x is loaded in 1024-t chunks (with a 1-column
    overlap for the t-1 tap) so the first matmuls start after ~1 MB of
    input DMA instead of the full x.
    """
    import concourse.mybir as mybir

    f32 = mybir.dt.float32
    bf16 = mybir.dt.bfloat16
    TCH = 128            # t per PSUM bank
    NSG = 16             # staging groups per co-pair
    SGT = T // NSG       # 256 t per staging group
    NB = SGT // TCH      # 2 banks per (staging group, r-half)
    XC = 1024            # t per x chunk
    ci = 0
    for g in range(2):
        wd_g = wd_ts[g]
        cur = [None, None]
        curq = -1
        for sg in range(NSG):
            qh = (sg * SGT) // XC
            if qh != curq:
                for gh in range(2):
                    q = 2 * g + gh
                    x_t = xpool.tile(
                        [128, 1 + XC], bf16,
                        tag=f"x{gh}{qh % 2}", name=f"x_t{gh}{qh % 2}",
                    )
                    nc.sync.dma_start(
                        out=x_t,
                        in_=x[128 * q : 128 * (q + 1), XC * qh : XC * qh + 1 + XC],
                    )
                    cur[gh] = x_t
                curq = qh
            y_t = ypool.tile([128, STRIDE * SGT], f32, tag="y", name="y_t")
            y_r = y_t.rearrange("p (t e) -> p t e", e=STRIDE)
            # Bank-outer order: each bank finishes its 16 matmuls
            # consecutively so its drain copy overlaps the next bank's
            # matmuls instead of all banks completing at once.
            for b in range(NB):
                tl = (sg * SGT + b * TCH) - XC * qh  # chunk-local t offset
                for h in range(2):
                    p_t = pspool.tile(
                        [128, 4 * TCH], f32,
                        tag=f"ps{b % 4}_{h}", name=f"p_t{b % 4}_{h}",
                    )
                    for rr in range(4):
                        r = 4 * h + rr
                        for tap in range(2):
                            for gh in range(2):
                                blk = (gh * 8 + r) * 2 + tap
                                lhsT = wd_g[:, blk * 64 : (blk + 1) * 64]
                                if preload:
                                    nc.tensor.ldweights(
                                        lhsT, tile_position=(0, 64 * gh)
                                    )
                                rhs = cur[gh][
                                    :, (1 - tap) + tl : (1 - tap) + tl + TCH
                                ]
                                mm = nc.tensor.matmul(
                                    p_t[64 * gh : 64 * (gh + 1), rr : 4 * TCH : 4],
                                    lhsT,
                                    rhs,
                                    start=(tap == 0),
                                    stop=(tap == 1),
                                    tile_position=(0, 64 * gh),
                                )
                                if preload:
                                    mm.ins.ldweights = False
                    dst = y_r[:, TCH * b : TCH * (b + 1), 4 * h : 4 * h + 4]
                    b_ap = bias_t[:, g : g + 1]
                    eng = pick(ci)
                    ci += 1
                    if eng is nc.scalar:
                        eng.add(dst, p_t, b_ap)
                    else:
                        eng.tensor_scalar_add(dst, p_t, b_ap)
                # ship this bank's 0.5 MB as soon as both r-halves landed,
                # keeping the final drain tail to one bank
                s0 = STRIDE * (SGT * sg + TCH * b)
                nc.sync.dma_start(
                    out=y[128 * g : 128 * (g + 1), s0 : s0 + STRIDE * TCH],
                    in_=y_t[:, STRIDE * TCH * b : STRIDE * TCH * (b + 1)],
                )


def _emit_g2(nc, tc, xpool, ypool, pspool, x, y, wd_t, bias_t, pick):
    import concourse.mybir as mybir

    f32 = mybir.dt.float32
    bf16 = mybir.dt.bfloat16
    TCH = 128            # t per PSUM bank
    NB = T // TCH        # 32 banks' worth of t-chunks per q
    BPS = 4              # banks per staging tile ([64, 4096] = 512 t)
    ci = 0
    for q in range(4):
        x_t = xpool.tile([128, 1 + T], bf16, tag="x", name="x_t")
        nc.sync.dma_start(out=x_t, in_=x[128 * q : 128 * (q + 1), :])
        for sg in range(NB // BPS):  # staging groups of 4 banks
            y_t = ypool.tile([64, STRIDE * TCH * BPS], f32, tag="y", name="y_t")
            p_ts = []
            for b in range(BPS):
                t0 = (sg * BPS + b) * TCH
                p_t = pspool.tile([128, 4 * TCH], f32, tag=f"ps{b}", name=f"p_t{b}")
                p_ts.append(p_t)
                for m in range(4):
                    blk = (q * 4 + m) * 2
                    for tap in range(2):
                        rhs = x_t[:, (1 - tap) + t0 : (1 - tap) + t0 + TCH]
                        nc.tensor.matmul(
                            p_t[:, m : 4 * TCH : 4],
                            wd_t[:, (blk + tap) * 128 : (blk + tap + 1) * 128],
                            rhs,
                            start=(tap == 0),
                            stop=(tap == 1),
                        )
            y_r = y_t.rearrange("p (t e) -> p t e", e=STRIDE)
            for b in range(BPS):
                for u in range(2):
                    # P[64u + co_l, 4*tl + m] -> y_t[co_l, 8*tl + 4u + m]
                    dst = y_r[:, TCH * b : TCH * (b + 1), 4 * u : 4 * u + 4]
                    src = p_ts[b][64 * u : 64 * (u + 1), :]
                    b_ap = bias_t[64 * u : 64 * (u + 1), q : q + 1]
                    eng = pick(ci)
                    ci += 1
                    if eng is nc.scalar:
                        eng.add(dst, src, b_ap)
                    else:
                        eng.tensor_scalar_add(dst, src, b_ap)
            nc.sync.dma_start(
                out=y[
                    64 * q : 64 * (q + 1),
                    STRIDE * TCH * BPS * sg : STRIDE * TCH * BPS * (sg + 1),
                ],
                in_=y_t,
            )


def _emit_h(nc, tc, xpool, ypool, pspool, x, y, wd_t, bias_t, pick):
    import concourse.mybir as mybir

    f32 = mybir.dt.float32
    bf16 = mybir.dt.bfloat16
    TWIN = 512
    NTWIN = T // TWIN  # 8
    ci = 0
    for ct in range(2):
        xj = []
        for j in range(2):
            x_t = xpool.tile([128, 1 + T], bf16, tag=f"x{j}", name=f"x_t{j}")
            src = x[256 * ct + j : 256 * ct + 256 : 2, :]
            nc.sync.dma_start(out=x_t, in_=src)
            xj.append(x_t)
        for twin in range(NTWIN):
            t0 = twin * TWIN
            y_t = ypool.tile([128, STRIDE * TWIN], f32, tag="y", name="y_t")
            for r in range(8):
                p_t = pspool.tile(
                    [128, TWIN], f32, tag=f"ps{r % 4}", name=f"p_t{r % 4}"
                )
                for j in range(2):
                    for tap in range(2):
                        k = r + 8 * tap
                        rhs = xj[j][:, (1 - tap) + t0 : (1 - tap) + t0 + TWIN]
                        col = ((ct * 2 + j) * 16 + k) * 128
                        nc.tensor.matmul(
                            p_t,
                            wd_t[:, col : col + 128],
                            rhs,
                            start=(tap == 0 and j == 0),
                            stop=(tap == 1 and j == 1),
                        )
                out_ap = y_t[:, r : STRIDE * TWIN : STRIDE]
                b_ap = bias_t[:, 2 * ct : 2 * ct + 1]
                eng = pick(ci)
                ci += 1
                if eng is nc.scalar:
                    eng.add(out_ap, p_t, b_ap)
                else:
                    eng.tensor_scalar_add(out_ap, p_t, b_ap)
            nc.sync.dma_start(
                out=y[
                    128 * ct : 128 * (ct + 1),
                    STRIDE * t0 : STRIDE * t0 + STRIDE * TWIN,
                ],
                in_=y_t,
            )


def _prep_weights(weight: np.ndarray, variant=None) -> np.ndarray:
    import ml_dtypes

    variant = variant or VARIANT
    w2 = weight.reshape(CIN, K).astype(np.float32)
    p = np.arange(128)
    if variant in ("qp4", "qp4p"):
        wd = np.zeros((128, 64 * 64), np.float32)
        for q in range(4):
            for r in range(8):
                for tap in range(2):
                    blk = (q * 8 + r) * 2 + tap
                    # col = co_l, nonzero rows p = 2*co_l + j
                    wd[p, blk * 64 + p // 2] = w2[128 * q + p, r + 8 * tap]
    elif variant == "g2":
        wd = np.zeros((128, 32 * 128), np.float32)
        for q in range(4):
            for m in range(4):
                for tap in range(2):
                    blk = (q * 4 + m) * 2 + tap
                    for u in range(2):
                        # col = 64u + co_l, nonzero rows p = 2*co_l + j
                        wd[p, blk * 128 + 64 * u + p // 2] = w2[
                            128 * q + p, m + 4 * u + 8 * tap
                        ]
    else:
        wd = np.zeros((128, 64 * 128), np.float32)
        for ct in range(2):
            for j in range(2):
                for k in range(K):
                    base = ((ct * 2 + j) * 16 + k) * 128
                    wd[p, base + p] = w2[256 * ct + 2 * p + j, k]
    return wd.astype(ml_dtypes.bfloat16)


def _prep_bias(bias: np.ndarray, variant=None) -> np.ndarray:
    variant = variant or VARIANT
    p = np.arange(128)
    b4 = np.zeros((128, 4), np.float32)
    if variant in ("qp4", "qp4p"):
        for g in range(2):
            b4[:, g] = bias[128 * g + p]
    elif variant == "g2":
        for q in range(4):
            b4[:, q] = bias[64 * q + p % 64]
    else:
        for ct in range(2):
            b4[:, 2 * ct] = bias[128 * ct + p]
    return b4


def _make_exec(nc):
    """Build a jitted 8-core SPMD callable for a Bass module."""
    import jax
    import concourse.mybir as mybir
    from concourse import bass2jax
    from jax.sharding import Mesh, PartitionSpec
    from jax.experimental.shard_map import shard_map

    bass2jax.install_neuronx_cc_hook()

    partition_name = nc.partition_id_tensor.name if nc.partition_id_tensor else None

    in_names = []
    out_names = []
    out_avals = []
    zero_outs = []
    for alloc in nc.m.functions[0].allocations:
        if not isinstance(alloc, mybir.MemoryLocationSet):
            continue
        name = alloc.memorylocations[0].name
        if alloc.kind == "ExternalInput":
            if name != partition_name:
                in_names.append(name)
        elif alloc.kind == "ExternalOutput":
            shape = tuple(alloc.tensor_shape)
            dtype = mybir.dt.np(alloc.dtype)
            out_names.append(name)
            out_avals.append(jax.core.ShapedArray(shape, dtype))
            zero_outs.append(np.zeros(shape, dtype))
    n_params = len(in_names)
    all_in_names = list(in_names) + list(out_names)
    if partition_name is not None:
        all_in_names.append(partition_name)

    def _body(*args):
        operands = list(args)
        if partition_name is not None:
            operands.append(bass2jax.partition_id_tensor())
        outs = bass2jax._bass_exec_p.bind(
            *operands,
            out_avals=tuple(out_avals),
            in_names=tuple(all_in_names),
            out_names=tuple(out_names),
            lowering_input_output_aliases=(),
            sim_require_finite=True,
            sim_require_nnan=True,
            nc=nc,
        )
        return tuple(outs)

    devices = jax.devices()[:NCORES]
    mesh = Mesh(np.asarray(devices), ("core",))
    n_outs = len(out_names)
    in_specs = (PartitionSpec("core"),) * (n_params + n_outs)
    out_specs = (PartitionSpec("core"),) * n_outs
    sharded = jax.jit(
        shard_map(
            _body, mesh=mesh, in_specs=in_specs, out_specs=out_specs, check_rep=False
        ),
        keep_unused=True,
    )
    concat_zeros = [
        np.zeros((NCORES * z.shape[0], *z.shape[1:]), z.dtype) for z in zero_outs
    ]
    return (sharded, in_names, out_names, out_avals, concat_zeros)


def _get_exec():
    if "exec" not in _CACHE:
        nc = _build_nc()
        _CACHE["nc"] = nc
        _CACHE["exec"] = _make_exec(nc)
    return _CACHE["exec"]


def _make_concat_inputs(x, weight, bias):
    """Per-core input dict -> concatenated global arrays (order = in_names)."""
    import ml_dtypes

    wd = _prep_weights(weight)
    bias4 = _prep_bias(bias)
    xp = np.zeros((NCORES, CIN, 1 + T), ml_dtypes.bfloat16)
    xp[:, :, 1:] = x.astype(ml_dtypes.bfloat16)
    per_core = {
        "x": xp.reshape(NCORES * CIN, 1 + T),
        "wd": np.concatenate([wd] * NCORES, axis=0),
        "bias": np.concatenate([bias4] * NCORES, axis=0),
    }
    return per_core


def kernel(x, weight, bias) -> np.ndarray:
    x = np.asarray(x, dtype=np.float32)
    weight = np.asarray(weight, dtype=np.float32)
    bias = np.asarray(bias, dtype=np.float32)

    sharded, in_names, out_names, out_avals, concat_zeros = _get_exec()
    per_core = _make_concat_inputs(x, weight, bias)
    concat_in = [per_core[name] for name in in_names]
    out_arrs = sharded(*concat_in, *concat_zeros)
    yi = out_names.index("y")
    out = np.asarray(out_arrs[yi]).reshape(NCORES, COUT, SOUT)
    return out.astype(np.float32)


# revision 25
# speedup vs baseline: 1.5090x; 1.0478x over previous
"""Causal ConvTranspose1d (grouped, stride 8) Trainium2 Bass kernel.

Problem (hardcoded):
  x      [8, 512, 4096]  f32
  weight [512, 16, 1]    f32
  bias   [256]           f32
  out    [8, 256, 32768] f32   (= [B, Cout, T*stride])

Math (derived from the reference grouped dilated conv):
  with w2 = weight.reshape(512, 16), cpg = 2, stride = 8, K = 16:
  y[b, co, 8*t + r] = sum_{j in 0..1} ( w2[2co+j, r]   * x[b, 2co+j, t]
                                      + w2[2co+j, r+8] * x[b, 2co+j, t-1] )
                      + bias[co]          (x[., -1] == 0)

Sharding: data-parallel over batch; one batch element per NeuronCore (8 cores).

Variant "g2" (default):
  - x is loaded bf16 in natural 128-channel slices q = 0..3 ([128, 1+4096],
    leading zero column for the t-1 tap).
  - Pair-packed stationary matrices [128, 128] bf16 with 2 nonzeros per
    column: column 64*u + co_l accumulates both input channels (j = 0, 1)
    of output channel 64*q + co_l for phase r = m + 4*u.  One matmul per
    (m, tap) covers both j terms -> half the PE column streams of the
    diagonal formulation.
  - Matmuls write PSUM with free-dim stride 4 (offset m), so a PSUM bank
    [128 = (u, co_l), 512 = (t, m)] holds 128 t x 8 phases with phases
    m = 0..3 interleaved; the drain copies then write 16B-contiguous runs
    (4 f32) per output-time step instead of isolated 4B elements.
  - Drain copies (+ bias add) are split across Scalar, Vector and GpSimd.
  - [64, 4096] f32 staging tiles are DMAed to HBM contiguously.

Variant "h": diagonal stationaries (1 nonzero/col, 128-wide PSUM,
phase-major) as the previous baseline, but bf16 inputs and the drain
copies split across all three engines.
"""

import numpy as np

B, CIN, COUT, K, T = 8, 512, 256, 16, 4096
STRIDE = 8
SOUT = T * STRIDE  # 32768
NCORES = 8
VARIANT = "qp4"

_CACHE = {}


def _build_nc(variant=None):
    import concourse.mybir as mybir
    from concourse import bacc
    from concourse.tile import TileContext

    f32 = mybir.dt.float32
    bf16 = mybir.dt.bfloat16
    variant = variant or VARIANT

    nc = bacc.Bacc(trn_type="TRN2", target_bir_lowering=False, debug=False)

    if variant in ("qp4", "qp4p"):
        # 64 pair stationaries [128, 64]: blk = (q*8 + r)*2 + tap
        WDCOLS = 64 * 64
    elif variant == "g2":
        # 32 pair stationaries: blk = (q*4 + m)*2 + tap, each [128, 128]
        WDCOLS = 32 * 128
    else:
        # 64 diag stationaries: blk = ((ct*2 + j)*16 + k), each [128, 128]
        WDCOLS = 64 * 128
    x = nc.dram_tensor("x", [CIN, 1 + T], bf16, kind="ExternalInput").ap()
    wd = nc.dram_tensor("wd", [128, WDCOLS], bf16, kind="ExternalInput").ap()
    bias = nc.dram_tensor("bias", [128, 4], f32, kind="ExternalInput").ap()
    y = nc.dram_tensor("y", [COUT, SOUT], f32, kind="ExternalOutput").ap()

    # weighted round-robin over the two PSUM-capable copy engines
    # (GpSimd/Pool cannot access PSUM).  Act @1.2GHz vs DVE @0.96GHz
    # -> 5:4 split balances their busy time.
    def engines(nc):
        pat = [nc.scalar, nc.vector] * 4 + [nc.scalar]

        def pick(i):
            return pat[i % len(pat)]

        return pick

    with TileContext(nc) as tc:
        ps_bufs = 2
        y_bufs = 3 if variant in ("qp4", "qp4p") else 3
        with (
            tc.tile_pool(name="const", bufs=1) as cpool,
            tc.tile_pool(name="xp", bufs=2) as xpool,
            tc.tile_pool(name="yp", bufs=y_bufs) as ypool,
            tc.tile_pool(name="ps", bufs=ps_bufs, space="PSUM") as pspool,
        ):
            bias_t = cpool.tile([128, 4], f32)
            nc.sync.dma_start(out=bias_t, in_=bias)
            pick = engines(nc)

            if variant in ("qp4", "qp4p"):
                # split wd per co-pair so the first matmul only waits on half
                wd_ts = []
                for g in range(2):
                    wd_g = cpool.tile([128, WDCOLS // 2], bf16, name=f"wd{g}")
                    nc.sync.dma_start(
                        out=wd_g, in_=wd[:, g * (WDCOLS // 2) : (g + 1) * (WDCOLS // 2)]
                    )
                    wd_ts.append(wd_g)
                _emit_qp4(
                    nc, tc, xpool, ypool, pspool, x, y, wd_ts, bias_t, pick,
                    preload=(variant == "qp4p"),
                )
            else:
                wd_t = cpool.tile([128, WDCOLS], bf16)
                nc.sync.dma_start(out=wd_t, in_=wd)
                if variant == "g2":
                    _emit_g2(nc, tc, xpool, ypool, pspool, x, y, wd_t, bias_t, pick)
                else:
                    _emit_h(nc, tc, xpool, ypool, pspool, x, y, wd_t, bias_t, pick)
    nc.compile()
    return nc


def _emit_qp4(nc, tc, xpool, ypool, pspool, x, y, wd_ts, bias_t, pick, preload=False):
    """q-pair col-tiled variant: PSUM partitions = 128 contiguous co.

    Two M=64 matmuls per (r, tap) — one per q-slice of the co-pair — at
    tile_position (0, 0) and (0, 64) fill a [128, 512] PSUM bank that
    holds 128 t x 4 phases (free = 4*tl + (r - 4h)).  Copies are 128-wide
    with 16B-contiguous runs; y staging spans 128 partitions so the DMA
    reads at full rate.  x is loaded in 1024-t chunks (with a 1-column
    overlap for the t-1 tap) so the first matmuls start after ~1 MB of
    input DMA instead of the full x.
    """
    import concourse.mybir as mybir

    f32 = mybir.dt.float32
    bf16 = mybir.dt.bfloat16
    TCH = 256            # t per PSUM bank (256 t x 2 phases)
    NSG = 16             # staging groups per co-pair
    SGT = T // NSG       # 256 t per staging group
    XC = 1024            # t per x chunk
    ci = 0
    for g in range(2):
        wd_g = wd_ts[g]
        cur = [None, None]
        curq = -1
        for sg in range(NSG):
            qh = (sg * SGT) // XC
            if qh != curq:
                for gh in range(2):
                    q = 2 * g + gh
                    x_t = xpool.tile(
                        [128, 1 + XC], bf16,
                        tag=f"x{gh}{qh % 2}", name=f"x_t{gh}{qh % 2}",
                    )
                    nc.sync.dma_start(
                        out=x_t,
                        in_=x[128 * q : 128 * (q + 1), XC * qh : XC * qh + 1 + XC],
                    )
                    cur[gh] = x_t
                curq = qh
            y_t = ypool.tile([128, STRIDE * SGT], f32, tag="y", name="y_t")
            y_r = y_t.rearrange("p (t e) -> p t e", e=STRIDE)
            tl = sg * SGT - XC * qh  # chunk-local t offset
            # Each bank holds 256 t x 2 phases (free = 2*tloc + rr,
            # r = 2*h2 + rr) -> N=256 matmuls, stride-2 out, 8B-run copies.
            for h2 in range(4):
                p_t = pspool.tile(
                    [128, 2 * TCH], f32, tag=f"ps{h2}", name=f"p_t{h2}"
                )
                for rr in range(2):
                    r = 2 * h2 + rr
                    for tap in range(2):
                        for gh in range(2):
                            blk = (gh * 8 + r) * 2 + tap
                            lhsT = wd_g[:, blk * 64 : (blk + 1) * 64]
                            if preload:
                                nc.tensor.ldweights(
                                    lhsT, tile_position=(0, 64 * gh)
                                )
                            rhs = cur[gh][
                                :, (1 - tap) + tl : (1 - tap) + tl + TCH
                            ]
                            mm = nc.tensor.matmul(
                                p_t[64 * gh : 64 * (gh + 1), rr : 2 * TCH : 2],
                                lhsT,
                                rhs,
                                start=(tap == 0),
                                stop=(tap == 1),
                                tile_position=(0, 64 * gh),
                            )
                            if preload:
                                mm.ins.ldweights = False
                dst = y_r[:, :, 2 * h2 : 2 * h2 + 2]
                b_ap = bias_t[:, g : g + 1]
                eng = pick(ci)
                ci += 1
                if eng is nc.scalar:
                    eng.add(dst, p_t, b_ap)
                else:
                    eng.tensor_scalar_add(dst, p_t, b_ap)
            nc.sync.dma_start(
                out=y[
                    128 * g : 128 * (g + 1),
                    STRIDE * SGT * sg : STRIDE * SGT * (sg + 1),
                ],
                in_=y_t,
            )


def _emit_g2(nc, tc, xpool, ypool, pspool, x, y, wd_t, bias_t, pick):
    import concourse.mybir as mybir

    f32 = mybir.dt.float32
    bf16 = mybir.dt.bfloat16
    TCH = 128            # t per PSUM bank
    NB = T // TCH        # 32 banks' worth of t-chunks per q
    BPS = 4              # banks per staging tile ([64, 4096] = 512 t)
    ci = 0
    for q in range(4):
        x_t = xpool.tile([128, 1 + T], bf16, tag="x", name="x_t")
        nc.sync.dma_start(out=x_t, in_=x[128 * q : 128 * (q + 1), :])
        for sg in range(NB // BPS):  # staging groups of 4 banks
            y_t = ypool.tile([64, STRIDE * TCH * BPS], f32, tag="y", name="y_t")
            p_ts = []
            for b in range(BPS):
                t0 = (sg * BPS + b) * TCH
                p_t = pspool.tile([128, 4 * TCH], f32, tag=f"ps{b}", name=f"p_t{b}")
                p_ts.append(p_t)
                for m in range(4):
                    blk = (q * 4 + m) * 2
                    for tap in range(2):
                        rhs = x_t[:, (1 - tap) + t0 : (1 - tap) + t0 + TCH]
                        nc.tensor.matmul(
                            p_t[:, m : 4 * TCH : 4],
                            wd_t[:, (blk + tap) * 128 : (blk + tap + 1) * 128],
                            rhs,
                            start=(tap == 0),
                            stop=(tap == 1),
                        )
            y_r = y_t.rearrange("p (t e) -> p t e", e=STRIDE)
            for b in range(BPS):
                for u in range(2):
                    # P[64u + co_l, 4*tl + m] -> y_t[co_l, 8*tl + 4u + m]
                    dst = y_r[:, TCH * b : TCH * (b + 1), 4 * u : 4 * u + 4]
                    src = p_ts[b][64 * u : 64 * (u + 1), :]
                    b_ap = bias_t[64 * u : 64 * (u + 1), q : q + 1]
                    eng = pick(ci)
                    ci += 1
                    if eng is nc.scalar:
                        eng.add(dst, src, b_ap)
                    else:
                        eng.tensor_scalar_add(dst, src, b_ap)
            nc.sync.dma_start(
                out=y[
                    64 * q : 64 * (q + 1),
                    STRIDE * TCH * BPS * sg : STRIDE * TCH * BPS * (sg + 1),
                ],
                in_=y_t,
            )


def _emit_h(nc, tc, xpool, ypool, pspool, x, y, wd_t, bias_t, pick):
    import concourse.mybir as mybir

    f32 = mybir.dt.float32
    bf16 = mybir.dt.bfloat16
    TWIN = 512
    NTWIN = T // TWIN  # 8
    ci = 0
    for ct in range(2):
        xj = []
        for j in range(2):
            x_t = xpool.tile([128, 1 + T], bf16, tag=f"x{j}", name=f"x_t{j}")
            src = x[256 * ct + j : 256 * ct + 256 : 2, :]
            nc.sync.dma_start(out=x_t, in_=src)
            xj.append(x_t)
        for twin in range(NTWIN):
            t0 = twin * TWIN
            y_t = ypool.tile([128, STRIDE * TWIN], f32, tag="y", name="y_t")
            for r in range(8):
                p_t = pspool.tile(
                    [128, TWIN], f32, tag=f"ps{r % 4}", name=f"p_t{r % 4}"
                )
                for j in range(2):
                    for tap in range(2):
                        k = r + 8 * tap
                        rhs = xj[j][:, (1 - tap) + t0 : (1 - tap) + t0 + TWIN]
                        col = ((ct * 2 + j) * 16 + k) * 128
                        nc.tensor.matmul(
                            p_t,
                            wd_t[:, col : col + 128],
                            rhs,
                            start=(tap == 0 and j == 0),
                            stop=(tap == 1 and j == 1),
                        )
                out_ap = y_t[:, r : STRIDE * TWIN : STRIDE]
                b_ap = bias_t[:, 2 * ct : 2 * ct + 1]
                eng = pick(ci)
                ci += 1
                if eng is nc.scalar:
                    eng.add(out_ap, p_t, b_ap)
                else:
                    eng.tensor_scalar_add(out_ap, p_t, b_ap)
            nc.sync.dma_start(
                out=y[
                    128 * ct : 128 * (ct + 1),
                    STRIDE * t0 : STRIDE * t0 + STRIDE * TWIN,
                ],
                in_=y_t,
            )


def _prep_weights(weight: np.ndarray, variant=None) -> np.ndarray:
    import ml_dtypes

    variant = variant or VARIANT
    w2 = weight.reshape(CIN, K).astype(np.float32)
    p = np.arange(128)
    if variant in ("qp4", "qp4p"):
        wd = np.zeros((128, 64 * 64), np.float32)
        for q in range(4):
            for r in range(8):
                for tap in range(2):
                    blk = (q * 8 + r) * 2 + tap
                    # col = co_l, nonzero rows p = 2*co_l + j
                    wd[p, blk * 64 + p // 2] = w2[128 * q + p, r + 8 * tap]
    elif variant == "g2":
        wd = np.zeros((128, 32 * 128), np.float32)
        for q in range(4):
            for m in range(4):
                for tap in range(2):
                    blk = (q * 4 + m) * 2 + tap
                    for u in range(2):
                        # col = 64u + co_l, nonzero rows p = 2*co_l + j
                        wd[p, blk * 128 + 64 * u + p // 2] = w2[
                            128 * q + p, m + 4 * u + 8 * tap
                        ]
    else:
        wd = np.zeros((128, 64 * 128), np.float32)
        for ct in range(2):
            for j in range(2):
                for k in range(K):
                    base = ((ct * 2 + j) * 16 + k) * 128
                    wd[p, base + p] = w2[256 * ct + 2 * p + j, k]
    return wd.astype(ml_dtypes.bfloat16)


def _prep_bias(bias: np.ndarray, variant=None) -> np.ndarray:
    variant = variant or VARIANT
    p = np.arange(128)
    b4 = np.zeros((128, 4), np.float32)
    if variant in ("qp4", "qp4p"):
        for g in range(2):
            b4[:, g] = bias[128 * g + p]
    elif variant == "g2":
        for q in range(4):
            b4[:, q] = bias[64 * q + p % 64]
    else:
        for ct in range(2):
            b4[:, 2 * ct] = bias[128 * ct + p]
    return b4


def _make_exec(nc):
    """Build a jitted 8-core SPMD callable for a Bass module."""
    import jax
    import concourse.mybir as mybir
    from concourse import bass2jax
    from jax.sharding import Mesh, PartitionSpec
    from jax.experimental.shard_map import shard_map

    bass2jax.install_neuronx_cc_hook()

    partition_name = nc.partition_id_tensor.name if nc.partition_id_tensor else None

    in_names = []
    out_names = []
    out_avals = []
    zero_outs = []
    for alloc in nc.m.functions[0].allocations:
        if not isinstance(alloc, mybir.MemoryLocationSet):
            continue
        name = alloc.memorylocations[0].name
        if alloc.kind == "ExternalInput":
            if name != partition_name:
                in_names.append(name)
        elif alloc.kind == "ExternalOutput":
            shape = tuple(alloc.tensor_shape)
            dtype = mybir.dt.np(alloc.dtype)
            out_names.append(name)
            out_avals.append(jax.core.ShapedArray(shape, dtype))
            zero_outs.append(np.zeros(shape, dtype))
    n_params = len(in_names)
    all_in_names = list(in_names) + list(out_names)
    if partition_name is not None:
        all_in_names.append(partition_name)

    def _body(*args):
        operands = list(args)
        if partition_name is not None:
            operands.append(bass2jax.partition_id_tensor())
        outs = bass2jax._bass_exec_p.bind(
            *operands,
            out_avals=tuple(out_avals),
            in_names=tuple(all_in_names),
            out_names=tuple(out_names),
            lowering_input_output_aliases=(),
            sim_require_finite=True,
            sim_require_nnan=True,
            nc=nc,
        )
        return tuple(outs)

    devices = jax.devices()[:NCORES]
    mesh = Mesh(np.asarray(devices), ("core",))
    n_outs = len(out_names)
    in_specs = (PartitionSpec("core"),) * (n_params + n_outs)
    out_specs = (PartitionSpec("core"),) * n_outs
    sharded = jax.jit(
        shard_map(
            _body, mesh=mesh, in_specs=in_specs, out_specs=out_specs, check_rep=False
        ),
        keep_unused=True,
    )
    concat_zeros = [
        np.zeros((NCORES * z.shape[0], *z.shape[1:]), z.dtype) for z in zero_outs
    ]
    return (sharded, in_names, out_names, out_avals, concat_zeros)


def _get_exec():
    if "exec" not in _CACHE:
        nc = _build_nc()
        _CACHE["nc"] = nc
        _CACHE["exec"] = _make_exec(nc)
    return _CACHE["exec"]


def _make_concat_inputs(x, weight, bias):
    """Per-core input dict -> concatenated global arrays (order = in_names)."""
    import ml_dtypes

    wd = _prep_weights(weight)
    bias4 = _prep_bias(bias)
    xp = np.zeros((NCORES, CIN, 1 + T), ml_dtypes.bfloat16)
    xp[:, :, 1:] = x.astype(ml_dtypes.bfloat16)
    per_core = {
        "x": xp.reshape(NCORES * CIN, 1 + T),
        "wd": np.concatenate([wd] * NCORES, axis=0),
        "bias": np.concatenate([bias4] * NCORES, axis=0),
    }
    return per_core


def kernel(x, weight, bias) -> np.ndarray:
    x = np.asarray(x, dtype=np.float32)
    weight = np.asarray(weight, dtype=np.float32)
    bias = np.asarray(bias, dtype=np.float32)

    sharded, in_names, out_names, out_avals, concat_zeros = _get_exec()
    per_core = _make_concat_inputs(x, weight, bias)
    concat_in = [per_core[name] for name in in_names]
    out_arrs = sharded(*concat_in, *concat_zeros)
    yi = out_names.index("y")
    out = np.asarray(out_arrs[yi]).reshape(NCORES, COUT, SOUT)
    return out.astype(np.float32)
